# revision 29
# baseline (speedup 1.0000x reference)
"""Trainium2 Bass kernel for nn_CDGMLinear (2-layer graph-learning GNN).

Math per layer (reference):
    g    = relu(x @ gl_w + gl_b)                      # [N, L]
    dist = sq[:,None] + sq[None,:] - 2 g g^T          # [N, N]
    adj  = sigmoid((1+temp) * (-dist) + (5+theta))    # [N, N]
    gnn  = x @ gnn_w + gnn_b                          # [N, D]
    out  = (adj @ gnn) / rowsum(adj)
Layer 1 output gets relu; then out head: softmax(x @ out_w + out_b).

Sharding: row-block over 8 cores (B = N/8 rows of the adjacency per core).
Each core computes adj^T tiles [j_tile=128, i in its block] so the message
matmul contracts j on the partition axis.  The N x N matrix never touches HBM.

v2 (vs baseline; sim 894 us -> 665 us, ACT-sigmoid-bound at 81%):
 - ACT does ONLY the N^2 sigmoid (the true engine floor: 512 instrs x
   ~1.04 us); every other elementwise op runs on DVE (relu/square/scale/
   normalize), with GPSIMD absorbing squares and half the layer-1-pass-0
   row-sum pair-adds (only where no collective occupies the Pool queue).
 - Row-sum accumulation uses a bf16 pair/quad/oct tree with an f32
   accumulate every 8 tiles instead of an f32 add per tile
   (DVE 2134 -> ~630 ns/jt).
 - The gnn bias never enters the message path: row-stochastic weights
   make the normalized bias contribution exactly +b, applied at finalize.
 - Layer-2 folds out_w into the gnn projection (rank 10, host-side
   gnn_w1 @ out_w) and appends a ones column: the message matmul emits
   logits AND the row-sum in one [j, 11] stationary, so layer-2 has no
   row-sum tree and the softmax head works on 10-wide tiles.
 - The ones rows of the adjacency stationary come from zero-padded gl
   weights + unit biases (relu(0*x+1)=1) -- no memsets.
 - The i-range runs in 2 passes over all j-tiles, so half of x1 is
   finalized and AllGathered at the layer-1 midpoint; layer-2 orders
   j-tiles gather-half-0 first, hiding both collectives under compute.
 - All prep is emitted via an explicit (k, closure) schedule interleaved
   into the main loops at first use, with prep PSUM tiles on a separate
   bank tag so prep never blocks the z-tile/sigmoid rotation.
 - PSUM ledger: z [128,1024]x2 (4 banks) + prep zP [128,512]x2 (2) +
   msg accumulator [128,1024]x1 (2) = 8 banks.

Precision: all O(N^2) matmuls bf16; the adj diagonal is deterministic
bf16(sigth) and corrected in f32 via corr = sigth*gnn_f32 -
bf16(sigth)*gnn_bf16 (re-derived bit-exactly from local x); layer-2's
fused rowsum also gets the sigth - bf16(sigth) diagonal correction.
"""
import numpy as np
import ml_dtypes

import concourse.bacc as bacc
import concourse.tile as tile
import concourse.mybir as mybir
from concourse.bass_utils import run_bass_kernel_spmd

F32 = mybir.dt.float32
BF16 = mybir.dt.bfloat16
Act = mybir.ActivationFunctionType
Alu = mybir.AluOpType
AX = mybir.AxisListType.X

N = 16384
D = 128
L = 64
NCORES = 8
B = N // NCORES          # 2048 rows per core
JT = N // 128            # 128 j-tiles
ICH = 1024               # i-columns per pass
NPASS = B // ICH         # 2 passes
NG = N // 512            # 32 j-groups (4 j-tiles each)
NOUT = 10

_NC_CACHE = {}


def build(t, th, sigth, bfsigth):
    nc = bacc.Bacc("TRN2", target_bir_lowering=False, debug=False,
                   num_devices=NCORES)
    delta = sigth - bfsigth

    ins = {}

    def di(name, shape, dt):
        ins[name] = nc.dram_tensor(name, shape, dt, kind="ExternalInput")
        return ins[name]

    di("x_bf", [D, N], BF16)
    di("xr_bf", [D, B], BF16)
    di("xr_f32", [D, B], F32)
    di("ident", [128, 128], BF16)
    di("identf", [128, 128], F32)
    for l in range(2):
        di(f"wgl{l}", [D, L + 2], BF16)
        di(f"glb{l}", [L + 2, 1], F32)
    di("wgn0", [D, D], BF16)
    di("wgn32_0", [D, D], F32)
    di("wgnb0", [D, 1], F32)
    di("w2bf", [D, NOUT + 1], BF16)     # wgn1 @ out_w, zero col 10
    di("w2f32", [D, NOUT], F32)
    di("c2aug", [1, NOUT + 1], F32)     # [gnn_b1 @ out_w, 1.0]
    di("c2col", [NOUT, 1], F32)
    di("outbT", [NOUT, 1], F32)
    y_ext = nc.dram_tensor("y", [B, NOUT], F32, kind="ExternalOutput")

    with tile.TileContext(nc) as tc:
        with (
            tc.tile_pool(name="sb", bufs=1) as sb,
            tc.tile_pool(name="sbl", bufs=2) as sbl,
            tc.tile_pool(name="zp", bufs=2, space="PSUM") as zp,
            tc.tile_pool(name="mp", bufs=1, space="PSUM") as mp,
            tc.tile_pool(name="dram", bufs=1, space="DRAM") as dram,
        ):
            def ld(name, shape, dt):
                tl = sb.tile(shape, dt, name=f"{name}_sb")
                nc.sync.dma_start(tl[:], ins[name][:, :])
                return tl

            ident = ld("ident", [128, 128], BF16)
            identf = ld("identf", [128, 128], F32)
            wgl = [ld(f"wgl{l}", [D, L + 2], BF16) for l in range(2)]
            glb = []
            for l in range(2):
                g_ = sb.tile([L + 2, 1], F32, name=f"glb{l}_sb")
                nc.sync.dma_start(g_[:], ins[f"glb{l}"][:, :])
                glb.append(g_)
            wgn0 = ld("wgn0", [D, D], BF16)
            wgn32_0 = ld("wgn32_0", [D, D], F32)
            wgnb0 = ld("wgnb0", [D, 1], F32)
            w2bf = ld("w2bf", [D, NOUT + 1], BF16)
            w2f32 = ld("w2f32", [D, NOUT], F32)
            c2aug = ld("c2aug", [1, NOUT + 1], F32)
            c2col = ld("c2col", [NOUT, 1], F32)
            outbT = ld("outbT", [NOUT, 1], F32)
            ones1f = sb.tile([1, 128], F32, name="ones1f")
            nc.vector.memset(ones1f[:], 1.0)
            ones64b = sb.tile([64, 1], BF16, name="ones64b")
            nc.vector.memset(ones64b[:], 1.0)
            ones64f = sb.tile([64, 1], F32, name="ones64f")
            nc.vector.memset(ones64f[:], 1.0)
            ones128f = sb.tile([128, 1], F32, name="ones128f")
            nc.vector.memset(ones128f[:], 1.0)

            # ---- input DMAs
            xr_bf = sb.tile([D, B], BF16, name="xr_bf0", tag="xr_bf")
            nc.sync.dma_start(xr_bf[:], ins["xr_bf"][:, :])
            xr_f0 = sb.tile([D, B], F32, name="xr_f0", tag="xr_f")
            nc.sync.dma_start(xr_f0[:], ins["xr_f32"][:, :])
            x_bf0 = sb.tile([D, N], BF16, name="x_bf0", tag="x_bf")
            for r in range(NCORES):
                nc.sync.dma_start(x_bf0[:, r * B:(r + 1) * B],
                                  ins["x_bf"][:, r * B:(r + 1) * B])

            # =========================================================
            # emit helpers
            # =========================================================
            def emit_g(l, xs, out_bf, name, rows=64):
                """out_bf[0:rows,:512] = relu(wgl^T xs + glb); rows 64:66 are
                1.0 via the zero-padded weight columns and unit biases."""
                gp = zp.tile([rows, 512], F32, name=f"gp_{name}", tag="zP")
                nc.tensor.matmul(gp[:], wgl[l][:, 0:rows], xs,
                                 start=True, stop=True)
                nc.vector.tensor_scalar(out_bf[0:rows, :], gp[:],
                                        glb[l][0:rows, :], 0.0,
                                        Alu.add, Alu.max)

            def emit_jgroup(l, g, xsrc, xoff, aug, gnnt, sqb, bcb_sb):
                """j-side prep for 512 cols at global j = 512*g."""
                emit_g(l, xsrc[:, xoff:xoff + 512], aug, f"j{l}_{g}",
                       rows=66)
                gsq = sbl.tile([64, 512], BF16, name=f"gsq{l}_{g}", tag="gsq", bufs=1)
                eng = nc.gpsimd if l == 0 else nc.vector
                eng.tensor_tensor(gsq[:], aug[0:64, :], aug[0:64, :],
                                  Alu.mult)
                sqp = zp.tile([128, 4], F32, name=f"sqp{l}_{g}", tag="zP")
                for q in range(4):
                    nc.tensor.matmul(sqp[:, q:q + 1],
                                     gsq[:, q * 128:(q + 1) * 128],
                                     ones64b[:], start=True, stop=True)
                nc.vector.tensor_scalar(sqb[:], sqp[:], -t, th,
                                        Alu.mult, Alu.add)
                if l == 0:
                    gp2 = zp.tile([128, 512], F32, name=f"gp2{l}_{g}", tag="zP")
                    for q in range(4):
                        nc.tensor.matmul(gp2[:, q * 128:(q + 1) * 128],
                                         xsrc[:, xoff + q * 128:xoff + (q + 1) * 128],
                                         wgn0[:], start=True, stop=True)
                    nc.vector.tensor_copy(gnnt[:], gp2[:])
                else:
                    gp2 = zp.tile([128, 44], F32, name=f"gp2{l}_{g}", tag="zP")
                    for q in range(4):
                        nc.tensor.matmul(gp2[:, q * 11:(q + 1) * 11],
                                         xsrc[:, xoff + q * 128:xoff + (q + 1) * 128],
                                         w2bf[:], start=True, stop=True)
                    nc.vector.tensor_tensor(
                        gnnt[:].rearrange("p (q n) -> p q n", n=11),
                        gp2[:].rearrange("p (q n) -> p q n", n=11),
                        bcb_sb[:].rearrange("p n -> p () n").broadcast_to(
                            [128, 4, 11]),
                        Alu.add)

            def emit_ihalf(l, p, gr_src, aug_mov, gsqr, name):
                """i-side moving operand for i in [p*ICH,(p+1)*ICH).
                gr_src(bc) -> [64, 512] bf16 g tile for 512-col chunk bc."""
                for bc in range(2):
                    cs = slice(bc * 512, (bc + 1) * 512)
                    go = gr_src(bc)
                    nc.vector.tensor_scalar(aug_mov[0:64, cs], go[0:64, :],
                                            2.0 * t, None, Alu.mult)
                    nc.vector.tensor_tensor(gsqr[:, cs], go[0:64, :],
                                            aug_mov[0:64, cs], Alu.mult)
                for bc in range(2):
                    cs = slice(bc * 512, (bc + 1) * 512)
                    sqi = zp.tile([1, 512], F32, name=f"sqi{name}_{bc}",
                                  tag="zP")
                    nc.tensor.matmul(sqi[:], ones64f[:], gsqr[:, cs],
                                     start=True, stop=True)
                    nsq = sbl.tile([1, 512], F32, name=f"nsq{name}_{bc}",
                                   tag="nsq", bufs=1)
                    nc.vector.tensor_scalar(nsq[:], sqi[:], -0.5, None,
                                            Alu.mult)
                    hi = sbl.tile([1, 512], BF16, name=f"hi{name}_{bc}",
                                  tag="hi", bufs=1)
                    nc.vector.tensor_copy(hi[:], nsq[:])
                    lo = sbl.tile([1, 512], F32, name=f"lo{name}_{bc}",
                                  tag="lo", bufs=1)
                    nc.vector.tensor_tensor(lo[:], nsq[:], hi[:], Alu.subtract)
                    lob = sbl.tile([1, 512], BF16, name=f"lob{name}_{bc}",
                                   tag="lob", bufs=1)
                    nc.vector.tensor_copy(lob[:], lo[:])
                    nc.sync.dma_start(aug_mov[64:65, cs], hi[:])
                    nc.sync.dma_start(aug_mov[65:66, cs], lob[:])

            def emit_main_pass(l, p, order, augs, aug_mov, gnnts, sqbs,
                               preps=None, tree_pool=False):
                """Sweep all j accumulating msg + rowsum for one i half.
                preps: sorted [(k_emit, closure)] interleaved into the loop
                so prep emission (and PE-queue order) tracks first use."""
                mrows = 128 if l == 0 else NOUT + 1
                msg = mp.tile([mrows, ICH], F32, name=f"msg{l}_{p}", tag="msg")
                racc = None
                if l == 0:
                    racc = sb.tile([128, ICH], F32, name=f"racc{l}_{p}",
                                   tag="racc", bufs=2)
                adjs = [None, None]
                pairs = [None, None]
                quads = [None, None]
                pend_msg = []
                pi = 0
                if preps:
                    while pi < len(preps) and preps[pi][0] <= 0:
                        preps[pi][1]()
                        pi += 1
                for k, jt in enumerate(order):
                    while preps and pi < len(preps) and preps[pi][0] <= k:
                        preps[pi][1]()
                        pi += 1
                    g, q = jt // 4, jt % 4
                    augst = augs[g][:, q * 128:(q + 1) * 128]
                    z = zp.tile([128, ICH], F32, name=f"z{l}_{p}_{k}", tag="z")
                    for h in range(2):
                        nc.tensor.matmul(z[:, h * 512:(h + 1) * 512], augst,
                                         aug_mov[:, h * 512:(h + 1) * 512],
                                         start=True, stop=True)
                    adj = sbl.tile([128, ICH], BF16, name=f"adj{l}_{p}_{k}",
                                   tag="adj", bufs=4)
                    nc.scalar.activation(adj[:], z[:], Act.Sigmoid,
                                         bias=sqbs[g][:, q:q + 1], scale=1.0)
                    st = gnnts[g]
                    w = 128 if l == 0 else 11
                    sts = st[:, q * w:(q + 1) * w]
                    for h in range(2):
                        hs = slice(h * 512, (h + 1) * 512)
                        nc.tensor.matmul(msg[:, hs], sts, adj[:, hs],
                                         start=(k == 0), stop=(k == JT - 1))
                    if l == 0:
                        adjs[k % 2] = adj
                        if k % 2 == 1:
                            pr = sbl.tile([128, ICH], BF16,
                                          name=f"pr{l}_{p}_{k}", tag="pair",
                                          bufs=2)
                            peng = nc.gpsimd if (tree_pool and
                                                 k % 4 == 1) else nc.vector
                            peng.tensor_tensor(pr[:], adjs[0][:],
                                               adjs[1][:], Alu.add)
                            pairs[(k // 2) % 2] = pr
                        if k % 4 == 3:
                            qd = sbl.tile([128, ICH], BF16,
                                          name=f"qd{l}_{p}_{k}", tag="quad",
                                          bufs=2)
                            nc.vector.tensor_tensor(qd[:], pairs[0][:],
                                                    pairs[1][:], Alu.add)
                            quads[(k // 4) % 2] = qd
                        if k % 8 == 7:
                            nc.vector.tensor_tensor(quads[0][:], quads[0][:],
                                                    quads[1][:], Alu.add)
                            if k == 7:
                                nc.vector.tensor_copy(racc[:], quads[0][:])
                            else:
                                nc.vector.tensor_tensor(racc[:], racc[:],
                                                        quads[0][:], Alu.add)
                if preps:
                    while pi < len(preps):
                        preps[pi][1]()
                        pi += 1
                return msg, racc

            def emit_fin1a(p, msg, corr, x1c):
                # read the msg psum FIRST so its (single-buffered) slot is
                # released for the next pass before the rowsum chain runs
                for h in range(2):
                    hs = slice(h * 512, (h + 1) * 512)
                    cs = slice(p * ICH + h * 512, p * ICH + (h + 1) * 512)
                    nc.vector.tensor_tensor(x1c[p][:, hs], msg[:, hs],
                                            corr[:, cs], Alu.add)

            def emit_fin1b(p, racc, x1c, x1bf):
                rsum = sbl.tile([1, ICH], F32, name=f"rsum0_{p}", tag="rsum", bufs=1)
                for h in range(2):
                    hs = slice(h * 512, (h + 1) * 512)
                    rs = zp.tile([1, 512], F32, name=f"rs0_{p}_{h}", tag="zP")
                    nc.tensor.matmul(rs[:], ones128f[:], racc[:, hs],
                                     start=True, stop=True)
                    nc.vector.tensor_copy(rsum[0:1, hs], rs[:])
                rcp = sbl.tile([1, ICH], F32, name=f"rcp0_{p}", tag="rcp", bufs=1)
                nc.vector.reciprocal(rcp[:], rsum[:])
                for h in range(2):
                    hs = slice(h * 512, (h + 1) * 512)
                    bc = zp.tile([128, 512], F32, name=f"bc0_{p}_{h}", tag="zP")
                    for qq in range(2):
                        nc.tensor.matmul(
                            bc[:, qq * 256:(qq + 1) * 256], ones1f[:],
                            rcp[0:1, h * 512 + qq * 256:h * 512 + (qq + 1) * 256],
                            start=True, stop=True)
                    nc.vector.tensor_tensor(x1c[p][:, hs], x1c[p][:, hs],
                                            bc[:], Alu.mult)
                    nc.vector.tensor_scalar(x1c[p][:, hs], x1c[p][:, hs],
                                            wgnb0[:], 0.0, Alu.add, Alu.max)
                nc.vector.tensor_copy(x1bf[p][:], x1c[p][:])

            lgts = [None, None]

            def emit_fin2a(p, msg, corr2):
                lgt = sb.tile([NOUT + 1, ICH], F32, name=f"lgt_{p}",
                              tag=f"x1c{p}", bufs=1)
                lgts[p] = lgt
                nc.vector.tensor_tensor(
                    lgt[:], msg[:], corr2[:, p * ICH:(p + 1) * ICH], Alu.add)
                rs2 = sbl.tile([1, ICH], F32, name=f"rs2_{p}", tag="rs2",
                               bufs=1)
                nc.sync.dma_start(rs2[:], lgt[NOUT:NOUT + 1, :])
                rcp = sbl.tile([1, ICH], F32, name=f"rcp1_{p}", tag="rcp", bufs=1)
                nc.vector.reciprocal(rcp[:], rs2[:])
                for h in range(2):
                    hs = slice(h * 512, (h + 1) * 512)
                    bc = zp.tile([NOUT, 512], F32, name=f"bc1_{p}_{h}",
                                 tag="zP")
                    nc.tensor.matmul(bc[:], ones1f[0:1, 0:NOUT],
                                     rcp[0:1, hs], start=True, stop=True)
                    nc.vector.tensor_tensor(lgt[0:NOUT, hs], lgt[0:NOUT, hs],
                                            bc[:], Alu.mult)

            def emit_fin2b(p):
                lgt = lgts[p]
                e = sbl.tile([NOUT, ICH], F32, name=f"e_{p}", tag="e", bufs=1)
                for h in range(2):
                    hs = slice(h * 512, (h + 1) * 512)
                    nc.scalar.activation(e[:, hs], lgt[0:NOUT, hs], Act.Exp,
                                         bias=outbT[:])
                y8 = sbl.tile([128, 8 * NOUT], F32, name=f"y8_{p}",
                              tag="y8", bufs=1)
                for it in range(ICH // 128):
                    tp = zp.tile([128, NOUT], F32, name=f"tpy_{p}_{it}",
                                 tag="zP")
                    nc.tensor.transpose(tp[:], e[:, it * 128:(it + 1) * 128],
                                        identf[0:NOUT, 0:NOUT])
                    eT = sbl.tile([128, NOUT], F32, name=f"eT_{p}_{it}",
                                  tag="eT")
                    nc.vector.tensor_copy(eT[:], tp[:])
                    es = sbl.tile([128, 1], F32, name=f"es_{p}_{it}", tag="es")
                    nc.vector.reduce_sum(es[:], eT[:], axis=AX)
                    rse = sbl.tile([128, 1], F32, name=f"rse_{p}_{it}",
                                   tag="rse")
                    nc.vector.reciprocal(rse[:], es[:])
                    nc.vector.tensor_scalar(y8[:, it * NOUT:(it + 1) * NOUT],
                                            eT[:], rse[:], None, Alu.mult)
                rows = slice(p * ICH, (p + 1) * ICH)
                nc.sync.dma_start(
                    y_ext[rows, :].rearrange("(it r) f -> r it f", it=8),
                    y8[:].rearrange("r (it f) -> r it f", f=NOUT))

            # =========================================================
            # LAYER 1 prep
            # =========================================================
            bcb_sb = None

            aug1 = [sb.tile([66, 512], BF16, name=f"aug0_{g}", tag=f"aug{g}")
                    for g in range(NG)]
            gnnt1 = [sb.tile([128, 512], BF16, name=f"gnnt0_{g}",
                             tag=f"gnt{g}") for g in range(NG)]
            sqb1 = [sb.tile([128, 4], F32, name=f"sqb0_{g}", tag=f"sqb{g}")
                    for g in range(NG)]
            # i-side: g of own rows from xr_bf (bit-identical to the j-side
            # values of the own block on this core)
            gr1 = [sb.tile([64, 512], BF16, name=f"gr0_{b}", tag=f"gr{b}")
                   for b in range(4)]
            for b in range(4):
                emit_g(0, xr_bf[:, b * 512:(b + 1) * 512], gr1[b], f"i0_{b}")
            aug_mov1 = [sb.tile([66, ICH], BF16, name=f"aug_mov0_{p}",
                                tag=f"am{p}") for p in range(NPASS)]
            gsqr1 = sb.tile([64, ICH], F32, name="gsqr0", tag="gsqr")
            emit_ihalf(0, 0, lambda bc: gr1[bc], aug_mov1[0], gsqr1, "0_0")
            corr1 = sb.tile([128, B], F32, name="corr0", tag="corr")

            def corr1_f32(bc):
                cs = slice(bc * 512, (bc + 1) * 512)
                gt = zp.tile([128, 512], F32, name=f"gt0_{bc}", tag="zP")
                nc.tensor.matmul(gt[:], wgn32_0[:], xr_f0[:, cs],
                                 start=True, stop=True)
                nc.vector.tensor_scalar(corr1[:, cs], gt[:], sigth, None,
                                        Alu.mult)

            def corr1_st(grp):
                gp4 = zp.tile([128, 512], F32, name=f"gp4_{grp}", tag="zP")
                for q in range(4):
                    bt = grp * 4 + q
                    nc.tensor.matmul(gp4[:, q * 128:(q + 1) * 128],
                                     xr_bf[:, bt * 128:(bt + 1) * 128],
                                     wgn0[:], start=True, stop=True)
                st = sbl.tile([128, 512], BF16, name=f"st0_{grp}", tag="st",
                              bufs=1)
                nc.vector.tensor_copy(st[:], gp4[:])
                for q in range(4):
                    bt = grp * 4 + q
                    tp = zp.tile([128, 128], BF16, name=f"tp0_{bt}", tag="zP")
                    nc.tensor.transpose(tp[:], st[:, q * 128:(q + 1) * 128],
                                        ident[:])
                    cs = slice(bt * 128, (bt + 1) * 128)
                    nc.vector.scalar_tensor_tensor(
                        corr1[:, cs], tp[:], -bfsigth, corr1[:, cs],
                        Alu.mult, Alu.add)

            # j-group prep closures (interleaved into pass-0 emission),
            # then deferred i-side pass-1 prep and corr1 chunks
            preps1 = [
                (4 * (g - 3),
                 (lambda g=g: emit_jgroup(0, g, x_bf0, g * 512, aug1[g],
                                          gnnt1[g], sqb1[g], bcb_sb)))
                for g in range(NG)]
            preps1.append((24, lambda: emit_ihalf(
                0, 1, lambda bc: gr1[2 + bc], aug_mov1[1], gsqr1, "0_1")))
            for bc in range(4):
                preps1.append((30 + 8 * bc, lambda bc=bc: corr1_f32(bc)))
            for grp in range(4):
                preps1.append((62 + 8 * grp, lambda grp=grp: corr1_st(grp)))
            preps1.sort(key=lambda x: x[0])

            # =========================================================
            # layer-2 persistent tiles
            # =========================================================
            aug2 = [sb.tile([66, 512], BF16, name=f"aug1_{g}", tag=f"aug{g}")
                    for g in range(NG)]
            gnnt2 = [sb.tile([128, 44], BF16, name=f"gnnt1_{g}",
                             tag=f"gnt{g}") for g in range(NG)]
            sqb2 = [sb.tile([128, 4], F32, name=f"sqb1_{g}", tag=f"sqb{g}")
                    for g in range(NG)]
            gr2 = [sb.tile([64, 512], BF16, name=f"gr1_{b}", tag=f"gr{b}")
                   for b in range(4)]
            aug_mov2 = [sb.tile([66, ICH], BF16, name=f"aug_mov1_{p}",
                                tag=f"am{p}") for p in range(NPASS)]
            gsqr2 = sb.tile([64, ICH], F32, name="gsqr1", tag="gsqr")
            corr2 = sb.tile([NOUT + 1, B], F32, name="corr1b", tag="corr2")
            bcb2_ps = zp.tile([128, NOUT + 1], F32, name="bcb1", tag="zP")
            nc.tensor.matmul(bcb2_ps[:], ones1f[:], c2aug[:, :],
                             start=True, stop=True)
            bcb2_sb = sb.tile([128, NOUT + 1], F32, name="bcb_sb1",
                              tag="bcb_sb2")
            nc.vector.tensor_copy(bcb2_sb[:], bcb2_ps[:])
            x_bf1 = sb.tile([D, N], BF16, name="x_bf1", tag="x_bf")
            x1c = [sb.tile([128, ICH], F32, name=f"x1c_{p}", tag=f"x1c{p}")
                   for p in range(NPASS)]
            x1bf = [sb.tile([128, ICH], BF16, name=f"x1bf_{p}", tag=f"x1b{p}")
                    for p in range(NPASS)]
            ag_in = [dram.tile([128, ICH], BF16, name=f"ag_in{p}")
                     for p in range(NPASS)]
            ag_out = [dram.tile([NCORES * 128, ICH], BF16, name=f"ag_out{p}",
                                addr_space="Shared") for p in range(NPASS)]

            # =========================================================
            # LAYER 1 main: 2 passes; AllGather each half when done
            # =========================================================
            def l2_iprep(p):
                """layer-2 i-side prep for half p (local x1)."""
                for b in range(2):
                    emit_g(1, x1bf[p][:, b * 512:(b + 1) * 512],
                           gr2[p * 2 + b], f"i1_{p}_{b}")
                emit_ihalf(1, p, lambda bc, p=p: gr2[p * 2 + bc],
                           aug_mov2[p], gsqr2, f"1_{p}")

            def l2_corr(p):
                """corr2 for half p: rowsum diag row, f32 part, stored part."""
                nc.vector.memset(corr2[0:NOUT + 1,
                                       p * ICH:(p + 1) * ICH], delta)
                for bc in range(2):
                    csl = slice(bc * 512, (bc + 1) * 512)
                    cs = slice(p * ICH + bc * 512, p * ICH + (bc + 1) * 512)
                    gt = zp.tile([NOUT, 512], F32, name=f"gt1_{p}_{bc}",
                                 tag="zP")
                    nc.tensor.matmul(gt[:], w2f32[:], x1c[p][:, csl],
                                     start=True, stop=True)
                    nc.vector.tensor_scalar(corr2[0:NOUT, cs], gt[:],
                                            c2col[:], sigth, Alu.add,
                                            Alu.mult)
                gp5 = st5 = None
                for bt in range(8):
                    if bt % 4 == 0:
                        gp5 = zp.tile([128, 44], F32, name=f"gp5_{p}_{bt}",
                                      tag="zP")
                        st5 = sbl.tile([128, 44], BF16, name=f"st5_{p}_{bt}",
                                       tag="st5")
                    q = bt % 4
                    nc.tensor.matmul(gp5[:, q * 11:(q + 1) * 11],
                                     x1bf[p][:, bt * 128:(bt + 1) * 128],
                                     w2bf[:], start=True, stop=True)
                    if q == 3:
                        nc.vector.tensor_tensor(
                            st5[:].rearrange("p (q n) -> p q n", n=11),
                            gp5[:].rearrange("p (q n) -> p q n", n=11),
                            bcb2_sb[:].rearrange("p n -> p () n").broadcast_to(
                                [128, 4, 11]),
                            Alu.add)
                        for qq in range(4):
                            bt2 = bt - 3 + qq
                            tp = zp.tile([NOUT, 128], BF16,
                                         name=f"tp1_{p}_{bt2}", tag="zP")
                            nc.tensor.transpose(
                                tp[:], st5[:, qq * 11:qq * 11 + NOUT],
                                ident[:])
                            cs = slice(p * ICH + bt2 * 128,
                                       p * ICH + (bt2 + 1) * 128)
                            nc.vector.scalar_tensor_tensor(
                                corr2[0:NOUT, cs], tp[:], -bfsigth,
                                corr2[0:NOUT, cs], Alu.mult, Alu.add)

            # layer-2 j-group prep closures (first-use order), then the
            # deferred i-side half-1 and corr2 work
            preps2 = []
            gi = 0
            for p in range(NPASS):
                for r in range(NCORES):
                    for gg in range(2):
                        g = r * 4 + p * 2 + gg
                        k_emit = 4 * (gi - 3) if gi < 16 else 4 * gi
                        preps2.append(
                            (k_emit,
                             (lambda g=g, r=r, p=p, gg=gg: emit_jgroup(
                                 1, g, x_bf1, r * B + p * ICH + gg * 512,
                                 aug2[g], gnnt2[g], sqb2[g], bcb2_sb))))
                        gi += 1
            preps2.append((44, lambda: l2_iprep(1)))
            preps2.append((54, lambda: l2_corr(0)))
            preps2.append((58, lambda: l2_corr(1)))
            preps2.sort(key=lambda x: x[0])

            order1 = list(range(JT))
            for p in range(NPASS):
                msg, racc = emit_main_pass(0, p, order1, aug1, aug_mov1[p],
                                           gnnt1, sqb1,
                                           preps=(preps1 if p == 0 else None),
                                           tree_pool=(p == 0))
                emit_fin1a(p, msg, corr1, x1c)
                emit_fin1b(p, racc, x1c, x1bf)
                nc.sync.dma_start(ag_in[p][:], x1bf[p][:])
                nc.gpsimd.collective_compute(
                    "AllGather", Alu.bypass,
                    ins=[ag_in[p].opt()],
                    outs=[ag_out[p].opt()],
                    replica_groups=[list(range(NCORES))],
                )
                # gathered x1 for this half -> x_bf1 columns of every block
                for r in range(NCORES):
                    nc.sync.dma_start(
                        x_bf1[:, r * B + p * ICH:r * B + (p + 1) * ICH],
                        ag_out[p][r * 128:(r + 1) * 128, :])
                if p == 0:
                    l2_iprep(0)
                else:
                    for gi_pre in range(6):
                        preps2[gi_pre][1]()


            # =========================================================
            # LAYER 2 main: j-tiles of gather-half 0 first
            # =========================================================
            order2 = [r * 16 + h * 8 + k
                      for h in range(2) for r in range(NCORES)
                      for k in range(8)]
            for p in range(NPASS):
                msg, _ = emit_main_pass(1, p, order2, aug2, aug_mov2[p],
                                        gnnt2, sqb2,
                                        preps=(preps2[6:] if p == 0 else None))
                emit_fin2a(p, msg, corr2)
            for p in range(NPASS):
                emit_fin2b(p)

    nc.compile()
    return nc


def _get_nc(t, th, sigth, bfsigth):
    key = (t, th, sigth, bfsigth)
    if _NC_CACHE.get("key") != key:
        _NC_CACHE["nc"] = build(t, th, sigth, bfsigth)
        _NC_CACHE["key"] = key
    return _NC_CACHE["nc"]


def _pad_gl(w):
    w = np.asarray(w, dtype=np.float32)
    out = np.zeros((D, L + 2), dtype=np.float32)
    out[:, :L] = w
    return out


def _pad_gb(b):
    out = np.ones((L + 2, 1), dtype=np.float32)
    out[:L, 0] = np.asarray(b, dtype=np.float32)
    return out


def kernel(feat_matrix, gl_w0, gl_b0, gl_w1, gl_b1,
           gnn_w0, gnn_b0, gnn_w1, gnn_b1,
           out_w, out_b, temp, theta,
           adj_matrix=None, get_item_index=None, set_index=None,
           val_index=None, mask_matrix=None, **_unused):
    bf = ml_dtypes.bfloat16
    f32 = np.float32

    x = np.ascontiguousarray(np.asarray(feat_matrix, dtype=f32))
    assert x.shape == (N, D)
    t = 1.0 + float(np.asarray(temp))
    th = 5.0 + float(np.asarray(theta))
    sigth = float(1.0 / (1.0 + np.exp(-np.float32(th))))
    bfsigth = float(np.float32(bf(np.float32(sigth))))
    lo16 = float(np.float32(bf(np.nextafter(np.float32(sigth), np.float32(0.0)))))
    hi16 = float(np.float32(bf(np.nextafter(np.float32(sigth), np.float32(1.0)))))
    assert lo16 == bfsigth == hi16, "sigth too close to a bf16 boundary"

    xT = np.ascontiguousarray(x.T)                       # [D, N] f32
    xT_bf = xT.astype(bf)

    wgn0 = np.ascontiguousarray(np.asarray(gnn_w0, dtype=f32))
    wgn1 = np.ascontiguousarray(np.asarray(gnn_w1, dtype=f32))
    ow = np.ascontiguousarray(np.asarray(out_w, dtype=f32))
    w2 = wgn1 @ ow                                       # [D, 10]
    c2 = np.asarray(gnn_b1, dtype=f32) @ ow              # [10]
    w2pad = np.zeros((D, NOUT + 1), dtype=f32)
    w2pad[:, :NOUT] = w2

    common = {
        "x_bf": xT_bf,
        "ident": np.eye(128, dtype=bf),
        "identf": np.eye(128, dtype=f32),
        "wgl0": _pad_gl(gl_w0).astype(bf),
        "glb0": _pad_gb(gl_b0),
        "wgl1": _pad_gl(gl_w1).astype(bf),
        "glb1": _pad_gb(gl_b1),
        "wgn0": wgn0.astype(bf),
        "wgn32_0": wgn0,
        "wgnb0": np.asarray(gnn_b0, dtype=f32).reshape(D, 1),
        "w2bf": w2pad.astype(bf),
        "w2f32": np.ascontiguousarray(w2),
        "c2aug": np.concatenate([c2, [1.0]]).astype(f32).reshape(1, NOUT + 1),
        "c2col": c2.reshape(NOUT, 1),
        "outbT": np.asarray(out_b, dtype=f32).reshape(NOUT, 1),
    }

    in_maps = []
    for c in range(NCORES):
        blk = slice(c * B, (c + 1) * B)
        m = dict(common)
        m["xr_bf"] = np.ascontiguousarray(xT_bf[:, blk])
        m["xr_f32"] = np.ascontiguousarray(xT[:, blk])
        in_maps.append(m)

    nc = _get_nc(t, th, sigth, bfsigth)
    res = run_bass_kernel_spmd(nc, in_maps, core_ids=list(range(NCORES)))
    return np.concatenate([res.results[c]["y"] for c in range(NCORES)], axis=0)


if __name__ == "__main__":
    import time
    t0 = time.time()
    nc = build(1.1, 5.1, 0.9939401149, 0.9921875)
    print(f"build+compile: {time.time() - t0:.1f}s")
